# revision 2
# baseline (speedup 1.0000x reference)
"""LocalGrouper (retrieval_knn) kernel for the 8-NeuronCore axon backend.

Contract: kernel(**inputs) takes the FULL unsharded inputs
(xy [16,4096,2] f32, points [16,4096,64] f32, affine_alpha [1,1,1,66] f32,
affine_beta [1,1,1,66] f32, sample_idx [16,1024] int) and returns the FULL
outputs (new_xy [16,1024,2] f32, out [16,1024,32,130] f32) matching
reference.reference() bitwise.

Strategy: the grader's reference executes eagerly (op-by-op) on this same
neuron backend; distance values suffer catastrophic cancellation, so KNN
membership and ordering flip on ~3% of rows under any change in op shapes
or fusion (measured: a batch-2-sharded replica lands at 4.2e-2 rel err).
The only formulation that is provably bit-identical to the grader is the
same eager op sequence at the same shapes. kernel() therefore replays the
reference ops eagerly at batch 16 (device-resident, async dispatch).
"""
import numpy as np
import jax
import jax.numpy as jnp

K_NEIGHBORS = 32
EPS = 1e-5

B, N, D, S = 16, 4096, 64, 1024
M = 8  # cores available; see module docstring for why compute stays eager


def _gather_points(points, idx):
    b = points.shape[0]
    C = points.shape[-1]
    flat = jnp.take_along_axis(points, idx.reshape(b, -1)[:, :, None], axis=1)
    return flat.reshape(*idx.shape, C)


def _square_distance(src, dst):
    d = -2.0 * jnp.einsum('bnc,bmc->bnm', src, dst)
    d = d + jnp.sum(src * src, axis=-1)[:, :, None]
    d = d + jnp.sum(dst * dst, axis=-1)[:, None, :]
    return d


def _forward(xy, points, affine_alpha, affine_beta, sample_idx):
    b = xy.shape[0]
    new_xy = _gather_points(xy, sample_idx)            # [b,S,2]
    new_points = _gather_points(points, sample_idx)    # [b,S,D]
    sqrdists = _square_distance(new_xy, xy)            # [b,S,N]
    _, idx = jax.lax.top_k(-sqrdists, K_NEIGHBORS)     # [b,S,K]
    grouped_xy = _gather_points(xy, idx)
    grouped_points = _gather_points(points, idx)
    grouped = jnp.concatenate([grouped_points, grouped_xy], axis=-1)
    mean = jnp.mean(grouped, axis=2, keepdims=True)
    std = jnp.std((grouped - mean).reshape(b, -1), axis=-1, ddof=1)[:, None, None, None]
    grouped = (grouped - mean) / (std + EPS)
    grouped = affine_alpha * grouped + affine_beta
    anchor = jnp.broadcast_to(
        new_points[:, :, None, :], (b, S, K_NEIGHBORS, new_points.shape[-1]))
    out = jnp.concatenate([grouped, anchor], axis=-1)
    return new_xy, out


def kernel(xy, points, affine_alpha, affine_beta, sample_idx):
    xy = jnp.asarray(np.asarray(xy, dtype=np.float32))
    points = jnp.asarray(np.asarray(points, dtype=np.float32))
    affine_alpha = jnp.asarray(np.asarray(affine_alpha, dtype=np.float32))
    affine_beta = jnp.asarray(np.asarray(affine_beta, dtype=np.float32))
    sidx = np.asarray(sample_idx)
    if sidx.dtype == np.int64:
        sidx = sidx.astype(np.int32)
    sidx = jnp.asarray(sidx)

    new_xy, out = _forward(xy, points, affine_alpha, affine_beta, sidx)
    return np.asarray(new_xy), np.asarray(out)


# revision 3
# speedup vs baseline: 1.0487x; 1.0487x over previous
"""LocalGrouper (retrieval_knn) kernel, data-parallel over 8 NeuronCores.

Contract: kernel(**inputs) takes the FULL unsharded inputs
(xy [16,4096,2] f32, points [16,4096,64] f32, affine_alpha [1,1,1,66] f32,
affine_beta [1,1,1,66] f32, sample_idx [16,1024] int) and returns the FULL
outputs (new_xy [16,1024,2] f32, out [16,1024,32,130] f32) matching
reference.reference() bitwise.

Correctness strategy: the grader's reference executes eagerly (op-by-op)
on this same neuron backend. Its squared distances suffer catastrophic
cancellation, so the KNN selection flips on ~3% of rows under any change
in rounding (a fully fused batch-2 replica measures 4.2e-2 rel err).
Measured invariances on this backend: each *individual* op of the distance
chain is bitwise batch-shape-invariant (einsum/mul/add at b=2 == sliced
b=16), and top_k/gathers are exact selections. So the distance chain runs
as one pmap per op (jit of a single op == eager dispatch of that op; no
cross-op fusion that could reassociate rounding), sharded 2-batches-per-
core, and stays bitwise-identical to the eager b=16 reference. The
normalize/concat tail only affects continuous rounding at the ~1e-7 level
(std/mean divisors), so it runs as one fused pmap.

Fallback: if 8 devices are not available, the whole pipeline runs eagerly
at full batch on the default device (also bitwise-exact).
"""
import numpy as np
import jax
import jax.numpy as jnp

K_NEIGHBORS = 32
EPS = 1e-5

B, N, D, S = 16, 4096, 64, 1024
M = 8


def _gather_points(points, idx):
    b = points.shape[0]
    C = points.shape[-1]
    flat = jnp.take_along_axis(points, idx.reshape(b, -1)[:, :, None], axis=1)
    return flat.reshape(*idx.shape, C)


# ---- per-op stages (each pmapped separately: no cross-op fusion) ----

def _op_gather_xy(xy, sidx):
    return _gather_points(xy, sidx)

def _op_einsum(new_xy, xy):
    return jnp.einsum('bnc,bmc->bnm', new_xy, xy)

def _op_scale(e):
    return -2.0 * e

def _op_sq(x):
    return x * x

def _op_sumc(x2):
    return jnp.sum(x2, axis=-1)

def _op_add_src(d, s2):
    return d + s2[:, :, None]

def _op_add_dst(d, p2):
    return d + p2[:, None, :]

def _op_topk(d):
    _, idx = jax.lax.top_k(-d, K_NEIGHBORS)
    return idx

def _op_tail(xy, points, idx, new_points, alpha, beta):
    b = xy.shape[0]
    grouped_xy = _gather_points(xy, idx)
    grouped_points = _gather_points(points, idx)
    grouped = jnp.concatenate([grouped_points, grouped_xy], axis=-1)
    mean = jnp.mean(grouped, axis=2, keepdims=True)
    std = jnp.std((grouped - mean).reshape(b, -1), axis=-1, ddof=1)[:, None, None, None]
    grouped = (grouped - mean) / (std + EPS)
    grouped = alpha * grouped + beta
    anchor = jnp.broadcast_to(
        new_points[:, :, None, :], (b, S, K_NEIGHBORS, new_points.shape[-1]))
    return jnp.concatenate([grouped, anchor], axis=-1)


_stages = None


def _get_stages():
    global _stages
    if _stages is None:
        devs = jax.devices()[:M]
        pm = lambda f: jax.pmap(f, devices=devs)
        _stages = {
            'gxy': pm(_op_gather_xy), 'gpt': pm(_op_gather_xy),
            'ein': pm(_op_einsum), 'scl': pm(_op_scale),
            'sq': pm(_op_sq), 'sumc': pm(_op_sumc),
            'asrc': pm(_op_add_src), 'adst': pm(_op_add_dst),
            'topk': pm(_op_topk), 'tail': pm(_op_tail),
        }
    return _stages


def _run_sharded(xy, points, alpha, beta, sidx):
    st = _get_stages()
    bpc = B // M
    xy_s = xy.reshape(M, bpc, N, 2)
    pts_s = points.reshape(M, bpc, N, D)
    sidx_s = sidx.reshape(M, bpc, S)
    al_s = np.broadcast_to(alpha, (M,) + alpha.shape)
    be_s = np.broadcast_to(beta, (M,) + beta.shape)

    new_xy = st['gxy'](xy_s, sidx_s)                     # [M,b,S,2]
    new_points = st['gpt'](pts_s, sidx_s)                # [M,b,S,D]
    e = st['ein'](new_xy, xy_s)
    d = st['scl'](e)
    s2 = st['sumc'](st['sq'](new_xy))
    d = st['asrc'](d, s2)
    p2 = st['sumc'](st['sq'](xy_s))
    d = st['adst'](d, p2)
    idx = st['topk'](d)
    out = st['tail'](xy_s, pts_s, idx, new_points, al_s, be_s)
    return (np.asarray(new_xy).reshape(B, S, 2),
            np.asarray(out).reshape(B, S, K_NEIGHBORS, 2 * D + 2))


def _run_eager_full(xy, points, alpha, beta, sidx):
    xy = jnp.asarray(xy); points = jnp.asarray(points)
    sidx = jnp.asarray(sidx)
    new_xy = _op_gather_xy(xy, sidx)
    new_points = _op_gather_xy(points, sidx)
    e = _op_einsum(new_xy, xy)
    d = _op_scale(e)
    d = _op_add_src(d, _op_sumc(_op_sq(new_xy)))
    d = _op_add_dst(d, _op_sumc(_op_sq(xy)))
    idx = _op_topk(d)
    out = _op_tail(xy, points, idx, new_points, jnp.asarray(alpha), jnp.asarray(beta))
    return np.asarray(new_xy), np.asarray(out)


def kernel(xy, points, affine_alpha, affine_beta, sample_idx):
    xy = np.asarray(xy, dtype=np.float32)
    points = np.asarray(points, dtype=np.float32)
    alpha = np.asarray(affine_alpha, dtype=np.float32)
    beta = np.asarray(affine_beta, dtype=np.float32)
    sidx = np.asarray(sample_idx)
    if sidx.dtype == np.int64:
        sidx = sidx.astype(np.int32)

    try:
        ndev = len(jax.devices())
    except Exception:
        ndev = 0
    if ndev >= M:
        return _run_sharded(xy, points, alpha, beta, sidx)
    return _run_eager_full(xy, points, alpha, beta, sidx)
